# revision 9
# baseline (speedup 1.0000x reference)
"""Trainium2 Bass kernel for nn_DFlashAttentionAccum (GQA attention block).

Sharding: tensor-parallel over heads across 8 cores. Core i owns kv-head i
and q-heads 4i..4i+3. Wq/Wk/Wv are column-sharded (output dim), caches are
sharded by kv-head. After attention each core AllGathers the (transposed)
per-head attention outputs and computes a 512-column shard of the final
o_proj output, which the host concatenates.

Numerics: fp16 inputs (host-cast) for all matmul operands, fp32 PSUM
accumulation, fp32 vector math for norms/softmax statistics. Softmax is
computed without a running max (scores are provably bounded here); a
constant bias of EXP_BIAS is folded into exp() for fp16 range safety and
cancels exactly in the normalization.
"""

import sys
from contextlib import ExitStack

sys.path.insert(0, "/opt/trn_rl_repo")

import numpy as np

import concourse.bass as bass
import concourse.bacc as bacc
import concourse.tile as tile
from concourse import masks, mybir
from concourse.bass_utils import run_bass_kernel_spmd

# Problem constants (hardcoded per contract)
HID = 4096
L = 128
CTX = 896
T = CTX + L            # 1024 new tokens (ctx + block)
STATE = 3072
S = STATE + T          # 4096 attend length
HQ = 32
HKV = 8
REP = HQ // HKV        # 4 q-heads per kv-head
D = 128
HALF = D // 2
EPS = 1e-6
SCALE = D ** -0.5
NCORES = 8
HT = HID // 128        # 32 contraction tiles
TT = T // 128          # 8 new-token tiles
ST = S // 128          # 32 attend tiles
CT_TILES = STATE // 128  # 24 cache tiles
EXP_BIAS = -4.0

F16 = mybir.dt.float16
F32 = mybir.dt.float32


def _build_program(mask_sids):
    """Build the single-core Bass program (run SPMD on 8 cores).

    mask_sids: sorted tuple of s-tile indices whose mask tile is not
    all-True; a multiplicative 0/1 fp16 mask is applied to those tiles.
    """
    nc = bacc.Bacc("TRN2", target_bir_lowering=False)

    cT_d = nc.dram_tensor("cT", [HID, T], F16, kind="ExternalInput")
    wqT_d = nc.dram_tensor("wqT", [HID, REP * D], F16, kind="ExternalInput")
    wkvT_d = nc.dram_tensor("wkvT", [HID, 2 * D], F16, kind="ExternalInput")
    woT_d = nc.dram_tensor("woT", [HID, 512], F16, kind="ExternalInput")
    ckT_d = nc.dram_tensor("ckT", [D, STATE], F16, kind="ExternalInput")
    cv_d = nc.dram_tensor("cv", [STATE, D], F16, kind="ExternalInput")
    coswq_d = nc.dram_tensor("coswq", [L, D], F32, kind="ExternalInput")
    sinwq_d = nc.dram_tensor("sinwq", [L, D], F32, kind="ExternalInput")
    coswk_d = nc.dram_tensor("coswk", [T, D], F32, kind="ExternalInput")
    sinwk_d = nc.dram_tensor("sinwk", [T, D], F32, kind="ExternalInput")
    nmask = len(mask_sids)
    if nmask:
        maskt_d = nc.dram_tensor("maskt", [nmask * 128, 128], F16, kind="ExternalInput")

    out_d = nc.dram_tensor("out_shard", [L, 512], F32, kind="ExternalOutput")
    knew_d = nc.dram_tensor("k_new", [T, D], F32, kind="ExternalOutput")
    vnew_d = nc.dram_tensor("v_new", [T, D], F32, kind="ExternalOutput")

    with ExitStack() as ctx:
        tc = ctx.enter_context(tile.TileContext(nc))
        const = ctx.enter_context(tc.tile_pool(name="const", bufs=1))
        work = ctx.enter_context(tc.tile_pool(name="work", bufs=3))
        stat = ctx.enter_context(tc.tile_pool(name="stat", bufs=4))
        pt_pool = ctx.enter_context(tc.tile_pool(name="ptp", bufs=3))
        pp_s = ctx.enter_context(tc.tile_pool(name="pp_s", bufs=2, space="PSUM"))
        pp_kv = ctx.enter_context(tc.tile_pool(name="pp_kv", bufs=2, space="PSUM"))
        pp_o = ctx.enter_context(tc.tile_pool(name="pp_o", bufs=4, space="PSUM"))
        dram = ctx.enter_context(tc.tile_pool(name="dram", bufs=1, space="DRAM"))

        # ---------------- constant / resident SBUF tiles ----------------
        ct_sb = const.tile([128, HT, T], F16)            # c^T, h on partitions
        cTr = cT_d[:].rearrange("(n p) t -> p n t", p=128)
        # x-block columns first so q-proj can start early
        nc.sync.dma_start(out=ct_sb[:, :, CTX:T], in_=cTr[:, :, CTX:T])
        nc.sync.dma_start(out=ct_sb[:, :, 0:448], in_=cTr[:, :, 0:448])
        nc.sync.dma_start(out=ct_sb[:, :, 448:CTX], in_=cTr[:, :, 448:CTX])

        wq_sb = const.tile([128, HT, 512], F16)
        wqr = wqT_d[:].rearrange("(n p) e -> p n e", p=128)
        nc.sync.dma_start(out=wq_sb[:, 0:16, :], in_=wqr[:, 0:16, :])
        nc.sync.dma_start(out=wq_sb[:, 16:32, :], in_=wqr[:, 16:32, :])
        wo_sb = const.tile([128, HT, 512], F16)
        wor = woT_d[:].rearrange("(n p) e -> p n e", p=128)
        nc.sync.dma_start(out=wo_sb[:, 0:16, :], in_=wor[:, 0:16, :])
        nc.sync.dma_start(out=wo_sb[:, 16:32, :], in_=wor[:, 16:32, :])
        wkv_sb = const.tile([128, HT, 2 * D], F16)
        nc.sync.dma_start(out=wkv_sb, in_=wkvT_d[:].rearrange("(n p) e -> p n e", p=128))

        kT_sb = const.tile([128, ST, D], F16)            # K_full^T: d on partitions
        nc.sync.dma_start(
            out=kT_sb[:, 0:CT_TILES, :],
            in_=ckT_d[:].rearrange("p (n d) -> p n d", d=128),
        )
        vaug_sb = const.tile([128, ST, 132], F16)        # V_full + ones col, s on partitions
        nc.sync.dma_start(
            out=vaug_sb[:, 0:CT_TILES, 0:D],
            in_=cv_d[:].rearrange("(n p) d -> p n d", p=128),
        )
        nc.vector.memset(vaug_sb[:, :, D : D + 1], 1.0)

        coswq_sb = const.tile([128, D], F32)
        nc.sync.dma_start(out=coswq_sb, in_=coswq_d[:])
        sinwq_sb = const.tile([128, D], F32)
        nc.sync.dma_start(out=sinwq_sb, in_=sinwq_d[:])
        coswk_sb = const.tile([128, TT, D], F32)
        nc.sync.dma_start(out=coswk_sb, in_=coswk_d[:].rearrange("(n p) d -> p n d", p=128))
        sinwk_sb = const.tile([128, TT, D], F32)
        nc.sync.dma_start(out=sinwk_sb, in_=sinwk_d[:].rearrange("(n p) d -> p n d", p=128))
        if nmask:
            mask_sb = const.tile([128, nmask, 128], F16)
            nc.sync.dma_start(
                out=mask_sb, in_=maskt_d[:].rearrange("(n p) l -> p n l", p=128)
            )

        identity = const.tile([128, 128], F16)
        masks.make_identity(nc, identity[:])
        biasm4 = const.tile([128, 1], F32)
        nc.vector.memset(biasm4, EXP_BIAS)
        epst = const.tile([128, 1], F32)
        nc.vector.memset(epst, EPS)

        knew_sb = const.tile([128, TT, D], F32)
        vnew_sb = const.tile([128, TT, D], F32)
        qT_all = const.tile([128, REP * D], F16)         # q^T for the 4 local heads
        oT_sb = const.tile([128, REP, D], F16)           # o^T for the 4 local heads
        oT_all = const.tile([128, HT, D], F16)           # gathered o^T, all 32 heads

        # ---------------- helpers ----------------
        def norm_rope(src_psum, cosw, sinw, kT_dst, f32_sb_dst):
            """ane_norm + RoPE (+norm weight folded into cosw/sinw on host).

            src_psum: [128, 128] fp32 psum slice (tokens on partitions).
            kT_dst: destination slice in a [d, t]-layout fp16 buffer (via PE
            transpose), or None.
            f32_sb_dst: SBUF AP to also receive the fp32 result, or None.
            Returns the fp16 [t, d] tile.
            """
            st = stat.tile([128, 8], F32, tag="bn")
            nc.vector.bn_stats(out=st[:, 0:6], in_=src_psum)
            nc.vector.bn_aggr(out=st[:, 6:8], in_=st[:, 0:6])
            sd = stat.tile([128, 1], F32, tag="sd")
            nc.scalar.activation(
                out=sd, in_=st[:, 7:8],
                func=mybir.ActivationFunctionType.Sqrt,
                bias=epst[:, 0:1],
            )
            rs = stat.tile([128, 1], F32, tag="rs")
            nc.vector.reciprocal(out=rs, in_=sd)
            # y1 = (x - mean) * rstd
            y1 = work.tile([128, D], F32, tag="y1")
            nc.vector.tensor_scalar(
                out=y1, in0=src_psum,
                scalar1=st[:, 6:7], scalar2=rs[:, 0:1],
                op0=mybir.AluOpType.subtract, op1=mybir.AluOpType.mult,
            )
            t1 = work.tile([128, D], F32, tag="t1")
            nc.vector.tensor_mul(out=t1, in0=y1, in1=cosw)
            t2 = work.tile([128, D], F32, tag="t2")
            nc.vector.tensor_mul(out=t2[:, 0:HALF], in0=y1[:, HALF:D], in1=sinw[:, 0:HALF])
            nc.vector.tensor_mul(out=t2[:, HALF:D], in0=y1[:, 0:HALF], in1=sinw[:, HALF:D])
            r16 = work.tile([128, D], F16, tag="r16")
            if f32_sb_dst is not None:
                nc.vector.tensor_add(out=f32_sb_dst, in0=t1, in1=t2)
                nc.scalar.copy(out=r16, in_=f32_sb_dst)
            else:
                nc.vector.tensor_add(out=r16, in0=t1, in1=t2)
            if kT_dst is not None:
                tp = pp_s.tile([128, D], F16, tag="sc", name="tp")
                nc.tensor.transpose(tp, r16, identity)
                nc.vector.tensor_copy(out=kT_dst, in_=tp)
            return r16

        # four psum accumulators, one per local head: [o | denom] (129 cols)
        po = [pp_o.tile([128, 129], F32, tag="po", name=f"po{i}") for i in range(REP)]

        def attend(s):
            ps_sT = pp_s.tile([128, 512], F32, tag="sc")
            nc.tensor.matmul(ps_sT, lhsT=kT_sb[:, s, :], rhs=qT_all[:], start=True, stop=True)
            pt = pt_pool.tile([128, 512], F16, tag="pt")
            nc.scalar.activation(
                out=pt, in_=ps_sT,
                func=mybir.ActivationFunctionType.Exp,
                bias=biasm4[:, 0:1], scale=float(SCALE),
            )
            if s in mask_sids:
                mi = mask_sids.index(s)
                for b in range(REP):
                    nc.vector.tensor_mul(
                        out=pt[:, 128 * b : 128 * (b + 1)],
                        in0=pt[:, 128 * b : 128 * (b + 1)],
                        in1=mask_sb[:, mi, :],
                    )
            for b in range(REP):
                nc.tensor.matmul(
                    po[b][:, 0:129],
                    lhsT=pt[:, 128 * b : 128 * (b + 1)],
                    rhs=vaug_sb[:, s, 0:129],
                    start=(s == 0), stop=(s == ST - 1),
                )

        # ---------------- Q projection ----------------
        ps_q = pp_s.tile([128, 512], F32, tag="sc")
        for h in range(HT):
            nc.tensor.matmul(
                ps_q, lhsT=ct_sb[:, h, CTX:T], rhs=wq_sb[:, h, :],
                start=(h == 0), stop=(h == HT - 1),
            )
        for b in range(REP):
            norm_rope(
                ps_q[:, 128 * b : 128 * (b + 1)], coswq_sb, sinwq_sb,
                kT_dst=qT_all[:, 128 * b : 128 * (b + 1)], f32_sb_dst=None,
            )

        # ---------------- KV projection interleaved with cache attention ----
        for t in range(TT):
            ps_kv = pp_kv.tile([128, 2 * D], F32, tag="kv")
            for h in range(HT):
                nc.tensor.matmul(
                    ps_kv, lhsT=ct_sb[:, h, 128 * t : 128 * (t + 1)], rhs=wkv_sb[:, h, :],
                    start=(h == 0), stop=(h == HT - 1),
                )
            nc.vector.tensor_copy(out=vnew_sb[:, t, :], in_=ps_kv[:, D : 2 * D])
            nc.vector.tensor_copy(out=vaug_sb[:, CT_TILES + t, 0:D], in_=vnew_sb[:, t, :])
            norm_rope(
                ps_kv[:, 0:D], coswk_sb[:, t, :], sinwk_sb[:, t, :],
                kT_dst=kT_sb[:, CT_TILES + t, :],
                f32_sb_dst=knew_sb[:, t, :],
            )
            # interleave 3 cache-attention tiles per kv t-tile
            for s in range(3 * t, 3 * (t + 1)):
                attend(s)

        # ---------------- remaining attention tiles ----------------
        for s in range(CT_TILES, ST):
            attend(s)

        # ---------------- k_new / v_new writeback (gated single DMAs) ----
        gk = stat.tile([128, TT], F32, tag="gk")
        nc.gpsimd.tensor_copy(out=gk, in_=knew_sb[:, :, 0:1])
        nc.gpsimd.dma_start(out=knew_d[:].rearrange("(n p) d -> p n d", p=128), in_=knew_sb[:])
        gv = stat.tile([128, TT], F32, tag="gv")
        nc.gpsimd.tensor_copy(out=gv, in_=vnew_sb[:, :, 0:1])
        nc.gpsimd.dma_start(out=vnew_d[:].rearrange("(n p) d -> p n d", p=128), in_=vnew_sb[:])

        # ---------------- normalize o, transpose, AllGather --------------
        for b in range(REP):
            rec = stat.tile([128, 1], F32, tag="rec")
            nc.vector.reciprocal(out=rec, in_=po[b][:, 128:129])
            o16 = work.tile([128, D], F16, tag="o16")
            nc.vector.tensor_scalar_mul(
                out=o16, in0=po[b][:, 0:128], scalar1=rec[:, 0:1]
            )
            tp = pp_s.tile([128, D], F16, tag="sc", name="tpo")
            nc.tensor.transpose(tp, o16, identity)
            nc.vector.tensor_copy(out=oT_sb[:, b, :], in_=tp)

        gag = stat.tile([128, REP], F16, tag="gag")
        nc.gpsimd.tensor_copy(out=gag, in_=oT_sb[:, :, 0:1])
        ag_in = dram.tile([REP * D, L], F16)
        ag_out = dram.tile([HQ * D, L], F16)
        nc.gpsimd.dma_start(out=ag_in[:].rearrange("(b p) l -> p b l", p=128), in_=oT_sb[:])
        nc.gpsimd.collective_compute(
            "AllGather",
            mybir.AluOpType.bypass,
            replica_groups=[list(range(NCORES))],
            ins=[ag_in[:]],
            outs=[ag_out[:]],
        )
        nc.gpsimd.dma_start(out=oT_all[:], in_=ag_out[:].rearrange("(n p) l -> p n l", p=128))

        # ---------------- o_proj (512-column shard) ----------------------
        ps_out = pp_s.tile([128, 512], F32, tag="sc")
        for e in range(HT):
            nc.tensor.matmul(
                ps_out, lhsT=oT_all[:, e, :], rhs=wo_sb[:, e, :],
                start=(e == 0), stop=(e == HT - 1),
            )
        out_sb = const.tile([128, 512], F32)
        nc.vector.tensor_copy(out=out_sb, in_=ps_out)
        go = stat.tile([128, 1], F32, tag="go")
        nc.gpsimd.tensor_copy(out=go, in_=out_sb[:, 0:1])
        nc.gpsimd.dma_start(out=out_d[:], in_=out_sb)

    nc.compile()
    return nc


_PROGRAM_CACHE = {}


def kernel(x, x_ctx, cos_q, sin_q, cos_k, sin_k, cache_K, cache_V, causal_mask,
           Wq, Wk, Wv, Wo, q_norm_w, k_norm_w):
    x = np.asarray(x, dtype=np.float32)
    x_ctx = np.asarray(x_ctx, dtype=np.float32)
    cos_q = np.asarray(cos_q, dtype=np.float32)[0, 0]
    sin_q = np.asarray(sin_q, dtype=np.float32)[0, 0]
    cos_k = np.asarray(cos_k, dtype=np.float32)[0, 0]
    sin_k = np.asarray(sin_k, dtype=np.float32)[0, 0]
    cache_K = np.asarray(cache_K, dtype=np.float32)
    cache_V = np.asarray(cache_V, dtype=np.float32)
    causal_mask = np.asarray(causal_mask).astype(bool)
    Wq = np.asarray(Wq, dtype=np.float32)
    Wk = np.asarray(Wk, dtype=np.float32)
    Wv = np.asarray(Wv, dtype=np.float32)
    Wo = np.asarray(Wo, dtype=np.float32)
    q_norm_w = np.asarray(q_norm_w, dtype=np.float32)
    k_norm_w = np.asarray(k_norm_w, dtype=np.float32)

    # ---- host-side prep (layout only) ----
    c = np.concatenate([x_ctx[0], x[0]], axis=0)               # (T, HID)
    cT = np.ascontiguousarray(c.T).astype(np.float16)          # (HID, T)

    # fold norm weights into rope tables:
    # rope(w*y) = (w*cos)*y + (rot_w*sin)*rot(y), sign folded into sin table
    def fold(cos_t, sin_t, w):
        cosw = (cos_t * w[None, :]).astype(np.float32)
        rot_w = np.concatenate([w[HALF:], w[:HALF]])
        sinw = (sin_t * rot_w[None, :]).astype(np.float32).copy()
        sinw[:, :HALF] *= -1.0
        return np.ascontiguousarray(cosw), np.ascontiguousarray(sinw)

    coswq, sinwq = fold(cos_q, sin_q, q_norm_w)
    coswk, sinwk = fold(cos_k, sin_k, k_norm_w)

    # mask tiles: transposed (s, l); skip all-True tiles
    maskT = np.ascontiguousarray(causal_mask[0, 0].T)          # (S, L)
    mask_sids = []
    mask_tiles = []
    for s in range(ST):
        tl = maskT[128 * s : 128 * (s + 1)]
        if not tl.all():
            mask_sids.append(s)
            mask_tiles.append(tl.astype(np.float16))
    mask_sids = tuple(mask_sids)
    maskt = np.concatenate(mask_tiles, axis=0) if mask_tiles else None

    key = mask_sids
    if key not in _PROGRAM_CACHE:
        _PROGRAM_CACHE[key] = _build_program(mask_sids)
    nc = _PROGRAM_CACHE[key]

    in_maps = []
    for i in range(NCORES):
        m = {
            "cT": cT,
            "wqT": np.ascontiguousarray(Wq[512 * i : 512 * (i + 1), :].T).astype(np.float16),
            "wkvT": np.ascontiguousarray(
                np.concatenate([Wk[128 * i : 128 * (i + 1), :].T,
                                Wv[128 * i : 128 * (i + 1), :].T], axis=1)
            ).astype(np.float16),
            "woT": np.ascontiguousarray(Wo[512 * i : 512 * (i + 1), :].T).astype(np.float16),
            "ckT": np.ascontiguousarray(cache_K[0, i].T).astype(np.float16),
            "cv": np.ascontiguousarray(cache_V[0, i]).astype(np.float16),
            "coswq": coswq,
            "sinwq": sinwq,
            "coswk": coswk,
            "sinwk": sinwk,
        }
        if maskt is not None:
            m["maskt"] = maskt
        in_maps.append(m)

    results = run_bass_kernel_spmd(nc, in_maps, list(range(NCORES))).results

    out = np.concatenate([results[i]["out_shard"] for i in range(NCORES)], axis=1)[None]
    k_new = np.stack([results[i]["k_new"] for i in range(NCORES)], axis=0)[None]
    v_new = np.stack([results[i]["v_new"] for i in range(NCORES)], axis=0)[None]
    return out.astype(np.float32), k_new.astype(np.float32), v_new.astype(np.float32)
